# revision 46
# baseline (speedup 1.0000x reference)
"""Trainium2 Bass kernel for nn_CCAR_11579231830663 (dense_transformer).

Data-parallel over batch: 16 samples -> 8 NeuronCores x 2 samples. Only the
global z-score stats of x_g and g cross cores (AllReduce of 4 scalar sums).

Precision scheme: the softmax over energy is a near-argmax (energy scale ~1e4,
winner gaps down to ~0.1), so every matmul feeding the energy needs ~fp32
accuracy, but hardware fp32 matmul costs 4 cycles/row. Instead every operand
is kept as an fp16 limb pair: A1 = fp16(A), A2 = fp16(A - A1). fp16 carries
11 significant bits, so A@B ~= A1@B1 + A1@B2 + A2@B1 gives ~2^-21 relative
accuracy from three 1-cycle/row fp16 passes: 4x faster than fp32 mode with
negligible precision loss for this problem. The pv/attention output path
tolerates a single fp16 pass. Weights and x are pre-split and pre-transposed
on the host; x_g and g limb pairs spill to DRAM between the residual phase
and the attention phase.

Energy algebra (z-score folded out): E = a*pq^T (xc gc^T) pk with
a = 1/(s_xg*s_g) folded into the softmax exp. MT = (g xg^T) is computed on
UNCENTERED data; the centering flows into Mp = MT^T pk as rank-1 corrections
(means x row/col sums of pk) applied at PSUM eviction, so only the final
softmax waits on the AllReduce.
"""
import sys
sys.path.insert(0, '/opt/trn_rl_repo')

import numpy as np
from contextlib import ExitStack

import concourse.bass as bass
import concourse.tile as tile
from concourse import mybir
from concourse.masks import make_identity
from concourse.bass_utils import run_bass_kernel_spmd

F32 = mybir.dt.float32
F16 = mybir.dt.float16
AF = mybir.ActivationFunctionType
ALU = mybir.AluOpType
AX = mybir.AxisListType

N_CORES = 8
B, C, W = 16, 512, 1024
SPC = B // N_CORES      # samples per core
CT = C // 128           # channel tiles
KT = W // 128           # width 128-tiles
EPS = 1e-5
NTOT = float(B * C * W)

MAGIC = 12582912.0       # 1.5*2^23 fp32 round-to-nearest-int magic
TWOPI = float(2 * np.pi)
INV2PI = float(1.0 / (2 * np.pi))

# ---------------------------------------------------------------------------
# antenv.axon_hooks is missing in this container; run_bass_kernel_spmd
# imports it when tracing is requested. Provide a stub.
import types as _types

if 'antenv.axon_hooks' not in sys.modules:
    _m = _types.ModuleType('antenv.axon_hooks')
    _h = [None]
    _m.set_axon_ntff_profile_hook = lambda h: _h.__setitem__(0, h)
    _m.get_axon_ntff_profile_hook = lambda: _h[0]
    sys.modules['antenv.axon_hooks'] = _m
    try:
        import antenv as _antenv
        _antenv.axon_hooks = _m
    except ImportError:
        pass

# ---------------------------------------------------------------------------
# walrus workaround: limited sync waits per instruction; split excess waits
# onto same-engine NOPs.
_uid = [0]


def _split_multiwait(nc, limit=1):
    for f in nc.m.functions:
        for bb in f.blocks:
            insts = list(bb.instructions)
            out = []
            changed = False
            for inst in insts:
                si = inst.sync_info
                waits = list(si.on_wait) if si is not None and si.on_wait \
                    else []
                if len(waits) > limit:
                    changed = True
                    excess, keep = waits[:-limit], waits[-limit:]
                    si.on_wait = keep
                    inst.sync_info = si
                    for i in range(0, len(excess), limit):
                        chunk = excess[i:i + limit]
                        _uid[0] += 1
                        nop = mybir.InstNoOp(
                            name=f"I-waitsplit-{_uid[0]}", ins=[], outs=[])
                        nop.engine = inst.engine
                        nop.sync_info = mybir.SyncInfo(
                            on_wait=chunk, on_update=[])
                        out.append(nop)
                out.append(inst)
            if changed:
                bb.instructions = out


# ---------------------------------------------------------------------------
def _emit(nc, tc, ctx, dram):
    V = nc.vector
    S = nc.scalar
    T = nc.tensor

    singles = ctx.enter_context(tc.tile_pool(name="singles", bufs=1))
    spool = ctx.enter_context(tc.tile_pool(name="spool", bufs=1))
    nrm = ctx.enter_context(tc.tile_pool(name="nrm", bufs=2))
    mm_psum = ctx.enter_context(
        tc.tile_pool(name="mm_psum", bufs=3, space="PSUM"))
    sm_psum = ctx.enter_context(
        tc.tile_pool(name="sm_psum", bufs=2, space="PSUM"))

    ident = singles.tile([128, 128], F32, name="ident")
    make_identity(nc, ident[:])
    identh = singles.tile([128, 128], F16, name="identh")
    V.tensor_copy(out=identh[:], in_=ident[:])

    ones128 = singles.tile([128, 1], F32, name="ones128")
    V.memset(ones128[:], 1.0)
    onesh = singles.tile([128, 1], F16, name="onesh")
    V.tensor_copy(out=onesh[:], in_=ones128[:])
    onek1f = singles.tile([1, 128], F32, name="onek1f")
    V.memset(onek1f[:], 1.0)
    onek1 = singles.tile([1, 128], F16, name="onek1")
    V.tensor_copy(out=onek1[:], in_=onek1f[:])

    zcol2 = singles.tile([128, 2], F32, name="zcol2")
    V.memset(zcol2[:], 0.0)
    negmagic = singles.tile([128, 1], F32, name="negmagic")
    V.memset(negmagic[:], -MAGIC)
    inv2pic = singles.tile([128, 1], F32, name="inv2pic")
    V.memset(inv2pic[:], INV2PI)

    def load_bias_cols(name):
        t = singles.tile([128, CT], F32, name=f"{name}_cols")
        src = dram[name].ap().rearrange("(t p) -> p t", p=128)
        nc.sync.dma_start(out=t[:], in_=src)
        return t

    rb1b = load_bias_cols("rb1")
    rb2b = load_bias_cols("rb2")
    qbb = load_bias_cols("qb")
    kbb = load_bias_cols("kb")

    # stats columns: 0 sum_xg, 1 sumsq_xg, 2 sum_g, 3 sumsq_g
    stats_block = spool.tile([128, 4], F32, name="stats_block")
    V.memset(stats_block[:], 0.0)

    rsxg = [spool.tile([128, CT], F32, name=f"rsxg_{s}")
            for s in range(SPC)]
    rsg = [spool.tile([128, CT], F32, name=f"rsg_{s}")
           for s in range(SPC)]

    def rstd_from_var(varcol):
        veps = nrm.tile([128, 1], F32, name="veps")
        V.tensor_scalar_add(out=veps[:], in0=varcol, scalar1=EPS)
        s0 = nrm.tile([128, 1], F32, name="s0")
        S.activation(s0[:], veps[:], AF.Sqrt)
        y0 = nrm.tile([128, 1], F32, name="y0")
        V.reciprocal(out=y0[:], in_=s0[:])
        t1 = nrm.tile([128, 1], F32, name="nt1")
        V.tensor_tensor(out=t1[:], in0=y0[:], in1=y0[:], op=ALU.mult)
        V.tensor_tensor(out=t1[:], in0=t1[:], in1=veps[:], op=ALU.mult)
        V.tensor_scalar(out=t1[:], in0=t1[:], scalar1=-0.5, scalar2=1.5,
                        op0=ALU.mult, op1=ALU.add)
        y1 = nrm.tile([128, 1], F32, name="ny1")
        V.tensor_tensor(out=y1[:], in0=y0[:], in1=t1[:], op=ALU.mult)
        return y1

    # ======================= R phase: residual block =======================
    with ExitStack() as rctx:
        cw = rctx.enter_context(tc.tile_pool(name="cw", bufs=1))
        padp = rctx.enter_context(tc.tile_pool(name="padp", bufs=8))
        rscr = rctx.enter_context(tc.tile_pool(name="rscr", bufs=3))
        rhf = rctx.enter_context(tc.tile_pool(name="rhf", bufs=2))

        rw = {}
        for cn in ("rw1", "rw2"):
            for li in ("1", "2"):
                rw[cn + li] = cw.tile([128, 6144], F16, name=cn + li)
                nc.sync.dma_start(out=rw[cn + li][:],
                                  in_=dram[cn + li + "T"].ap())

        def wsl(wt, k, ci, co):
            base = ((k * CT + ci) * CT + co) * 128
            return wt[:, base:base + 128]

        def conv3(dst_cb, src1, src2, cn):
            """3-pass conv: w1*x1 + w1*x2 + w2*x1; dst_cb(co_t, ps)."""
            for co_t in range(CT):
                ps = mm_psum.tile([128, W], F32, name="mm_ps")
                for jc in range(2):
                    idx = 0
                    for wt, xt in ((rw[cn + "1"], src1),
                                   (rw[cn + "1"], src2),
                                   (rw[cn + "2"], src1)):
                        for k in range(3):
                            for ci_t in range(CT):
                                T.matmul(ps[:, jc * 512:(jc + 1) * 512],
                                         lhsT=wsl(wt, k, ci_t, co_t),
                                         rhs=xt[ci_t][:, jc * 512 + k + 1:
                                                      jc * 512 + k + 513],
                                         start=(idx == 0), stop=(idx == 35))
                                idx += 1
                dst_cb(co_t, ps)

        def inorm_sin(ps, bias_col, sin_dst, sin_accum=None):
            """sin_dst <- sin(instance_norm(ps + bias))."""
            t = rscr.tile([128, W], F32, name="rsA")
            S.activation(t[:], ps[:], AF.Identity, bias=bias_col)
            st = nrm.tile([128, 2, 6], F32, name="bn_st")
            V.bn_stats(st[:, 0, :], t[:, 0:512])
            V.bn_stats(st[:, 1, :], t[:, 512:1024])
            mv = nrm.tile([128, 2], F32, name="bn_mv")
            V.bn_aggr(mv[:], st[:])
            rstd = rstd_from_var(mv[:, 1:2])
            nmr = nrm.tile([128, 1], F32, name="nmr")
            V.tensor_tensor(out=nmr[:], in0=mv[:, 0:1], in1=rstd[:],
                            op=ALU.mult)
            V.tensor_scalar_mul(out=nmr[:], in0=nmr[:], scalar1=-1.0)
            w = rscr.tile([128, W], F32, name="rsB")
            S.activation(w[:], t[:], AF.Identity, bias=nmr[:],
                         scale=rstd[:])
            u = rscr.tile([128, W], F32, name="rsA")
            S.activation(u[:], w[:], AF.Identity, bias=negmagic[:],
                         scale=inv2pic[:])
            V.tensor_scalar_add(out=u[:], in0=u[:], scalar1=MAGIC)
            V.scalar_tensor_tensor(out=u[:], in0=u[:], scalar=-TWOPI,
                                   in1=w[:], op0=ALU.mult, op1=ALU.add)
            S.activation(sin_dst, u[:], AF.Sin, accum_out=sin_accum)

        for s in range(SPC):
            xp1 = [padp.tile([128, W + 4], F16, name="pad1")
                   for _ in range(CT)]
            xp2 = [padp.tile([128, W + 4], F16, name="pad2")
                   for _ in range(CT)]
            for c in range(CT):
                for t_ in (xp1[c], xp2[c]):
                    V.tensor_copy(out=t_[:, 0:2], in_=zcol2[:])
                    V.tensor_copy(out=t_[:, W + 2:W + 4], in_=zcol2[:])
                nc.sync.dma_start(
                    out=xp1[c][:, 2:W + 2],
                    in_=dram["x1"].ap()[s, c * 128:(c + 1) * 128, :])
                nc.sync.dma_start(
                    out=xp2[c][:, 2:W + 2],
                    in_=dram["x2"].ap()[s, c * 128:(c + 1) * 128, :])

            g11 = [padp.tile([128, W + 4], F16, name="pad1")
                   for _ in range(CT)]
            g12 = [padp.tile([128, W + 4], F16, name="pad2")
                   for _ in range(CT)]
            for c in range(CT):
                for t_ in (g11[c], g12[c]):
                    V.tensor_copy(out=t_[:, 0:2], in_=zcol2[:])
                    V.tensor_copy(out=t_[:, W + 2:W + 4], in_=zcol2[:])

            def c1_cb(co_t, ps):
                sf = rscr.tile([128, W], F32, name="rsB")
                inorm_sin(ps, rb1b[:, co_t:co_t + 1], sf[:])
                V.tensor_copy(out=g11[co_t][:, 2:W + 2], in_=sf[:])
                V.tensor_tensor(out=g12[co_t][:, 2:W + 2], in0=sf[:],
                                in1=g11[co_t][:, 2:W + 2],
                                op=ALU.subtract)

            conv3(c1_cb, xp1, xp2, "rw1")

            def c2_cb(co_t, ps):
                sl = slice(co_t * 128, (co_t + 1) * 128)
                sf = rscr.tile([128, W], F32, name="rsB")
                inorm_sin(ps, rb2b[:, co_t:co_t + 1], sf[:],
                          sin_accum=rsg[s][:, co_t:co_t + 1])
                # spill g limb pair to DRAM
                gp1 = rhf.tile([128, W], F16, name="gp1")
                V.tensor_copy(out=gp1[:], in_=sf[:])
                nc.sync.dma_start(out=dram["g_1"].ap()[s, sl, :],
                                  in_=gp1[:])
                gp2 = rhf.tile([128, W], F16, name="gp2")
                V.tensor_tensor(out=gp2[:], in0=sf[:], in1=gp1[:],
                                op=ALU.subtract)
                nc.sync.dma_start(out=dram["g_2"].ap()[s, sl, :],
                                  in_=gp2[:])
                gs2 = nrm.tile([128, 1], F32, name="gs2")
                sq = rscr.tile([128, W], F32, name="rsA")
                V.scalar_tensor_tensor(out=sq[:], in0=sf[:], scalar=0.0,
                                       in1=sf[:], op0=ALU.add,
                                       op1=ALU.mult, accum_out=gs2[:])
                # x_g = (x1 + x2) + g ; spill its limb pair
                xt = rscr.tile([128, W], F32, name="rsA")
                V.tensor_tensor(out=xt[:], in0=xp1[co_t][:, 2:W + 2],
                                in1=xp2[co_t][:, 2:W + 2], op=ALU.add)
                V.scalar_tensor_tensor(out=xt[:], in0=xt[:], scalar=0.0,
                                       in1=sf[:], op0=ALU.add, op1=ALU.add,
                                       accum_out=rsxg[s][:, co_t:co_t + 1])
                xs2 = nrm.tile([128, 1], F32, name="xs2")
                sq2 = rscr.tile([128, W], F32, name="rsB")
                V.scalar_tensor_tensor(out=sq2[:], in0=xt[:], scalar=0.0,
                                       in1=xt[:], op0=ALU.add,
                                       op1=ALU.mult, accum_out=xs2[:])
                xg1 = rhf.tile([128, W], F16, name="gp1")
                V.tensor_copy(out=xg1[:], in_=xt[:])
                nc.sync.dma_start(out=dram["xg_1"].ap()[s, sl, :],
                                  in_=xg1[:])
                xg2 = rhf.tile([128, W], F16, name="gp2")
                V.tensor_tensor(out=xg2[:], in0=xt[:], in1=xg1[:],
                                op=ALU.subtract)
                nc.sync.dma_start(out=dram["xg_2"].ap()[s, sl, :],
                                  in_=xg2[:])
                # global stats accumulation
                V.tensor_tensor(out=stats_block[:, 0:1],
                                in0=stats_block[:, 0:1],
                                in1=rsxg[s][:, co_t:co_t + 1], op=ALU.add)
                V.tensor_tensor(out=stats_block[:, 1:2],
                                in0=stats_block[:, 1:2], in1=xs2[:],
                                op=ALU.add)
                V.tensor_tensor(out=stats_block[:, 2:3],
                                in0=stats_block[:, 2:3],
                                in1=rsg[s][:, co_t:co_t + 1], op=ALU.add)
                V.tensor_tensor(out=stats_block[:, 3:4],
                                in0=stats_block[:, 3:4], in1=gs2[:],
                                op=ALU.add)

            conv3(c2_cb, g11, g12, "rw2")

    # ====================== AllReduce of the 4 sums ========================
    ps4 = mm_psum.tile([128, W], F32, name="mm_ps")
    T.matmul(ps4[:1, 0:4], lhsT=ones128[:], rhs=stats_block[:],
             start=True, stop=True)
    cc_sb = spool.tile([1, 4], F32, name="cc_sb")
    V.tensor_copy(out=cc_sb[:], in_=ps4[:1, 0:4])
    nc.sync.dma_start(out=dram["cc_in"].ap(), in_=cc_sb[:])
    nc.gpsimd.collective_compute(
        "AllReduce", ALU.add,
        replica_groups=[list(range(N_CORES))],
        ins=[dram["cc_in"].ap()],
        outs=[dram["cc_out"].ap()],
    )
    gstat = spool.tile([128, 4], F32, name="gstat")
    bcast = bass.AP(tensor=dram["cc_out"], offset=0, ap=[[0, 128], [1, 4]])
    nc.sync.dma_start(out=gstat[:], in_=bcast)

    def mean_rs(s1col, s2col, tag):
        m = spool.tile([128, 1], F32, name=f"m_{tag}")
        V.tensor_scalar_mul(out=m[:], in0=s1col, scalar1=1.0 / NTOT)
        t = spool.tile([128, 1], F32, name=f"v_{tag}")
        V.tensor_tensor(out=t[:], in0=s1col, in1=m[:], op=ALU.mult)
        V.tensor_scalar_mul(out=t[:], in0=t[:], scalar1=-1.0)
        V.tensor_tensor(out=t[:], in0=t[:], in1=s2col, op=ALU.add)
        V.tensor_scalar_mul(out=t[:], in0=t[:], scalar1=1.0 / (NTOT - 1.0))
        sq = spool.tile([128, 1], F32, name=f"sq_{tag}")
        S.activation(sq[:], t[:], AF.Sqrt)
        y0 = spool.tile([128, 1], F32, name=f"y0_{tag}")
        V.reciprocal(out=y0[:], in_=sq[:])
        t2 = spool.tile([128, 1], F32, name=f"t2_{tag}")
        V.tensor_tensor(out=t2[:], in0=y0[:], in1=y0[:], op=ALU.mult)
        V.tensor_tensor(out=t2[:], in0=t2[:], in1=t[:], op=ALU.mult)
        V.tensor_scalar(out=t2[:], in0=t2[:], scalar1=-0.5, scalar2=1.5,
                        op0=ALU.mult, op1=ALU.add)
        V.tensor_tensor(out=t2[:], in0=y0[:], in1=t2[:], op=ALU.mult)
        return m, t2

    m_xg, rs_xg = mean_rs(gstat[:, 0:1], gstat[:, 1:2], "xg")
    m_g, rs_g = mean_rs(gstat[:, 2:3], gstat[:, 3:4], "g")
    alpha = spool.tile([128, 1], F32, name="alpha")
    V.tensor_tensor(out=alpha[:], in0=rs_xg[:], in1=rs_g[:], op=ALU.mult)
    negalpha = spool.tile([128, 1], F32, name="negalpha")
    V.tensor_scalar_mul(out=negalpha[:], in0=alpha[:], scalar1=-1.0)
    negmg = spool.tile([128, 1], F32, name="negmg")
    V.tensor_scalar_mul(out=negmg[:], in0=m_g[:], scalar1=-1.0)
    negmx = spool.tile([128, 1], F32, name="negmx")
    V.tensor_scalar_mul(out=negmx[:], in0=m_xg[:], scalar1=-1.0)
    wmxmg = spool.tile([128, 1], F32, name="wmxmg")
    V.tensor_tensor(out=wmxmg[:], in0=m_xg[:], in1=m_g[:], op=ALU.mult)
    V.tensor_scalar_mul(out=wmxmg[:], in0=wmxmg[:], scalar1=float(W))

    # =================== P/M/E phases, one sample at a time ================
    with ExitStack() as ectx:
        pscr = ectx.enter_context(tc.tile_pool(name="pscr", bufs=3))
        psing = ectx.enter_context(tc.tile_pool(name="psing", bufs=1))
        vb_bc = psing.tile([128, C], F32, name="vb_bc")
        nc.sync.dma_start(out=vb_bc[:],
                          in_=bass.AP(tensor=dram["vb"], offset=0,
                                      ap=[[0, 128], [1, C]]))
        qk = psing.tile([128, 4 * 2048], F16, name="qkw")
        for i, nm in enumerate(["qw1T", "qw2T", "kw1T", "kw2T"]):
            nc.sync.dma_start(out=qk[:, i * 2048:(i + 1) * 2048],
                              in_=dram[nm].ap())
        vwt = psing.tile([128, 2048], F16, name="vwt")
        nc.sync.dma_start(out=vwt[:], in_=dram["vwT"].ap())

        def qsl(i, ci, co):
            base = i * 2048 + ci * 512 + co * 128
            return qk[:, base:base + 128]

        def split_pair(src_ap, l1, l2):
            """l1 <- fp16(src); l2 <- fp16(src - l1)."""
            V.tensor_copy(out=l1, in_=src_ap)
            V.tensor_tensor(out=l2, in0=src_ap, in1=l1, op=ALU.subtract)

        for s in range(SPC):
            with ExitStack() as sctx:
                # right-side stack: pools dying mid-sample
                hctx = sctx.enter_context(ExitStack())
                mtp = hctx.enter_context(
                    tc.tile_pool(name="mtp", bufs=1, side="right"))
                pkp = hctx.enter_context(
                    tc.tile_pool(name="pkp", bufs=1, side="right"))

                MT_1, MT_2 = [], []
                pq_1, pq_2 = [], []
                pk_1, pk_2 = [], []
                pvT = []
                pqp = sctx.enter_context(tc.tile_pool(name="pqp", bufs=1))
                pvp = sctx.enter_context(tc.tile_pool(name="pvp", bufs=1))
                with ExitStack() as tctx:
                    trio = tctx.enter_context(
                        tc.tile_pool(name="trio", bufs=1, side="right"))
                    xg_1, xg_2, g_1, g_2 = [], [], [], []
                    for dname, l1s, l2s, tag in (
                            ("xg", xg_1, xg_2, "x"), ("g", g_1, g_2, "g")):
                        for c in range(CT):
                            sl = slice(c * 128, (c + 1) * 128)
                            for li, ls in ((1, l1s), (2, l2s)):
                                t = trio.tile([128, W], F16,
                                              name=f"{tag}{li}_{c}")
                                nc.sync.dma_start(
                                    out=t[:],
                                    in_=dram[f"{dname}_{li}"]
                                    .ap()[s, sl, :])
                                ls.append(t)

                    # ---- transposed limbs, then MT (uncentered) ----
                    t2ctx = tctx.enter_context(ExitStack())
                    tp = t2ctx.enter_context(tc.tile_pool(name="tp",
                                                          bufs=1))

                    def transposed(src1, src2, tag):
                        t1s, t2s = [], []
                        for kt in range(KT):
                            ksl = slice(kt * 128, (kt + 1) * 128)
                            t1 = tp.tile([128, C], F16, name=f"{tag}1{kt}")
                            t2 = tp.tile([128, C], F16, name=f"{tag}2{kt}")
                            for ci in range(CT):
                                csl = slice(ci * 128, (ci + 1) * 128)
                                ps = sm_psum.tile([128, 128], F16,
                                                  name="smph")
                                T.transpose(ps[:], src1[ci][:, ksl],
                                            identh[:])
                                V.tensor_copy(out=t1[:, csl], in_=ps[:])
                                ps2 = sm_psum.tile([128, 128], F16,
                                                   name="smph")
                                T.transpose(ps2[:], src2[ci][:, ksl],
                                            identh[:])
                                V.tensor_copy(out=t2[:, csl], in_=ps2[:])
                            t1s.append(t1)
                            t2s.append(t2)
                        return t1s, t2s

                    xgT_1, xgT_2 = transposed(xg_1, xg_2, "xT")
                    ggT_1, ggT_2 = transposed(g_1, g_2, "gT")

                    for cpt in range(CT):
                        ps = mm_psum.tile([128, W], F32, name="mm_ps")
                        idx = 0
                        for (gt, xt) in ((ggT_1, xgT_1), (ggT_1, xgT_2),
                                         (ggT_2, xgT_1)):
                            for kt in range(KT):
                                T.matmul(
                                    ps[:, 0:C],
                                    lhsT=gt[kt][:, cpt * 128:
                                                (cpt + 1) * 128],
                                    rhs=xt[kt][:],
                                    start=(idx == 0), stop=(idx == 23))
                                idx += 1
                        l1 = mtp.tile([128, C], F16, name=f"MT1{cpt}")
                        l2 = mtp.tile([128, C], F16, name=f"MT2{cpt}")
                        split_pair(ps[:, 0:C], l1[:], l2[:])
                        MT_1.append(l1)
                        MT_2.append(l2)
                    t2ctx.close()  # free transposed limbs; keep naturals

                    # ---- pq / pk (3-pass projections), pvT (fp16) ----
                    def proj3(iw, src1, src2, bias_cols, prefix, pool,
                              o1, o2):
                        for co_t in range(CT):
                            ps = mm_psum.tile([128, W], F32, name="mm_ps")
                            for jc in range(2):
                                idx = 0
                                for wi, xt in ((iw, src1), (iw, src2),
                                               (iw + 1, src1)):
                                    for ci_t in range(CT):
                                        T.matmul(
                                            ps[:, jc * 512:(jc + 1) * 512],
                                            lhsT=qsl(wi, ci_t, co_t),
                                            rhs=xt[ci_t][:, jc * 512:
                                                         (jc + 1) * 512],
                                            start=(idx == 0),
                                            stop=(idx == 11))
                                        idx += 1
                            t = pscr.tile([128, W], F32, name="psA")
                            S.activation(t[:], ps[:], AF.Identity,
                                         bias=bias_cols[:, co_t:co_t + 1])
                            l1 = pool.tile([128, W], F16,
                                           name=f"{prefix}1{co_t}")
                            l2 = pool.tile([128, W], F16,
                                           name=f"{prefix}2{co_t}")
                            split_pair(t[:], l1[:], l2[:])
                            o1.append(l1)
                            o2.append(l2)

                    proj3(0, xg_1, xg_2, qbb, "pq", pqp, pq_1, pq_2)
                    proj3(2, g_1, g_2, kbb, "pk", pkp, pk_1, pk_2)
                    for kt in range(KT):
                        ps = mm_psum.tile([128, W], F32, name="mm_ps")
                        for ci_t in range(CT):
                            T.matmul(
                                ps[:, 0:C],
                                lhsT=g_1[ci_t][:, kt * 128:(kt + 1) * 128],
                                rhs=vwt[:, ci_t * 512:(ci_t + 1) * 512],
                                start=(ci_t == 0), stop=(ci_t == CT - 1))
                        t = pvp.tile([128, C], F16, name=f"pvT{kt}")
                        V.scalar_tensor_tensor(
                            out=t[:], in0=ps[:, 0:C], scalar=0.0,
                            in1=vb_bc[:], op0=ALU.add, op1=ALU.add)
                        pvT.append(t)
                # natural limb pairs freed here

                # ---- Mp = MT^T pk + rank-1 centering corrections ----
                # cs[j] = colsum pk ; u[j] = sum_c' rsg[c'] pk[c',j]
                Mp_1, Mp_2 = [], []
                mpp = sctx.enter_context(tc.tile_pool(name="mpp", bufs=1))
                with ExitStack() as mctx:
                    mrow = mctx.enter_context(
                        tc.tile_pool(name="mrow", bufs=1, side="right"))
                    rsch = []
                    for cpt in range(CT):
                        rc = mrow.tile([128, 1], F16, name=f"rsch{cpt}")
                        V.tensor_copy(out=rc[:],
                                      in_=rsg[s][:, cpt:cpt + 1])
                        rsch.append(rc)
                    psr = mm_psum.tile([128, W], F32, name="mm_ps")
                    psu = mm_psum.tile([128, W], F32, name="mm_ps")
                    for jc in range(2):
                        sl = slice(jc * 512, (jc + 1) * 512)
                        idx = 0
                        for pkt in (pk_1, pk_2):
                            for cpt in range(CT):
                                T.matmul(psr[:1, sl], lhsT=onesh[:],
                                         rhs=pkt[cpt][:, sl],
                                         start=(idx == 0), stop=(idx == 7))
                                T.matmul(psu[:1, sl],
                                         lhsT=rsch[cpt][:],
                                         rhs=pkt[cpt][:, sl],
                                         start=(idx == 0), stop=(idx == 7))
                                idx += 1
                    csrow = mrow.tile([1, W], F16, name="csrow")
                    V.tensor_copy(out=csrow[:], in_=psr[:1, :])
                    urow = mrow.tile([1, W], F16, name="urow")
                    V.tensor_copy(out=urow[:], in_=psu[:1, :])
                    # broadcast both rows to [128, W]
                    psb = mm_psum.tile([128, W], F32, name="mm_ps")
                    psb2 = mm_psum.tile([128, W], F32, name="mm_ps")
                    for jc in range(2):
                        sl = slice(jc * 512, (jc + 1) * 512)
                        T.matmul(psb[:, sl], lhsT=onek1[:],
                                 rhs=csrow[:, sl], start=True, stop=True)
                        T.matmul(psb2[:, sl], lhsT=onek1[:],
                                 rhs=urow[:, sl], start=True, stop=True)
                    cs_bc = mrow.tile([128, W], F32, name="cs_bc")
                    V.tensor_copy(out=cs_bc[:], in_=psb[:, :])
                    # vcomb = -m_x*u + W*m_x*m_g*cs (same for all rows)
                    vcomb = mrow.tile([128, W], F32, name="vcomb")
                    V.tensor_scalar(out=vcomb[:], in0=cs_bc[:],
                                    scalar1=wmxmg[:], scalar2=None,
                                    op0=ALU.mult, op1=ALU.bypass)
                    V.scalar_tensor_tensor(out=vcomb[:], in0=psb2[:, :],
                                           scalar=negmx[:], in1=vcomb[:],
                                           op0=ALU.mult, op1=ALU.add)

                    for ct in range(CT):
                        ps = mm_psum.tile([128, W], F32, name="mm_ps")
                        for jc in range(2):
                            idx = 0
                            for (mt, pkt) in ((MT_1, pk_1), (MT_1, pk_2),
                                              (MT_2, pk_1)):
                                for cpt in range(CT):
                                    T.matmul(
                                        ps[:, jc * 512:(jc + 1) * 512],
                                        lhsT=mt[cpt][:, ct * 128:
                                                    (ct + 1) * 128],
                                        rhs=pkt[cpt][:, jc * 512:
                                                    (jc + 1) * 512],
                                        start=(idx == 0), stop=(idx == 11))
                                    idx += 1
                        # Mp_c = ps - m_g*rsx[c]*cs + vcomb
                        ngr = nrm.tile([128, 1], F32, name="ngr")
                        V.tensor_tensor(out=ngr[:], in0=negmg[:],
                                        in1=rsxg[s][:, ct:ct + 1],
                                        op=ALU.mult)
                        t1 = pscr.tile([128, W], F32, name="psA")
                        V.tensor_scalar(out=t1[:], in0=cs_bc[:],
                                        scalar1=ngr[:], scalar2=None,
                                        op0=ALU.mult, op1=ALU.bypass)
                        V.tensor_tensor(out=t1[:], in0=t1[:],
                                        in1=vcomb[:], op=ALU.add)
                        V.tensor_tensor(out=t1[:], in0=t1[:],
                                        in1=ps[:], op=ALU.add)
                        l1 = mpp.tile([128, W], F16, name=f"Mp1{ct}")
                        l2 = mpp.tile([128, W], F16, name=f"Mp2{ct}")
                        split_pair(t1[:], l1[:], l2[:])
                        Mp_1.append(l1)
                        Mp_2.append(l2)

                # ---- energy -> softmax -> att^T (3-pass E) ----
                hctx.close()  # free MT + pk before E allocates attT
                attp = sctx.enter_context(tc.tile_pool(name="attp",
                                                       bufs=1))
                escr = sctx.enter_context(tc.tile_pool(name="escr",
                                                       bufs=2))
                attT = [attp.tile([128, W], F16, name=f"attT_{kt}")
                        for kt in range(KT)]
                for it in range(KT):
                    ps = mm_psum.tile([128, W], F32, name="mm_ps")
                    for jc in range(2):
                        idx = 0
                        for (pqt, mpt) in ((pq_1, Mp_1), (pq_1, Mp_2),
                                           (pq_2, Mp_1)):
                            for ct in range(CT):
                                T.matmul(
                                    ps[:, jc * 512:(jc + 1) * 512],
                                    lhsT=pqt[ct][:, it * 128:
                                                (it + 1) * 128],
                                    rhs=mpt[ct][:, jc * 512:
                                               (jc + 1) * 512],
                                    start=(idx == 0), stop=(idx == 11))
                                idx += 1
                    rowmax = nrm.tile([128, 1], F32, name="rowmax")
                    V.tensor_reduce(out=rowmax[:], in_=ps[:], axis=AX.X,
                                    op=ALU.max)
                    nb = nrm.tile([128, 1], F32, name="negb")
                    V.tensor_tensor(out=nb[:], in0=rowmax[:],
                                    in1=negalpha[:], op=ALU.mult)
                    e = pscr.tile([128, W], F32, name="psA")
                    rowsum = nrm.tile([128, 1], F32, name="rowsum")
                    S.activation(e[:], ps[:], AF.Exp, bias=nb[:],
                                 scale=alpha[:], accum_out=rowsum[:])
                    rs = nrm.tile([128, 1], F32, name="rs")
                    V.reciprocal(out=rs[:], in_=rowsum[:])
                    er = escr.tile([128, W], F16, name="psR")
                    V.tensor_scalar_mul(out=er[:], in0=e[:], scalar1=rs[:])
                    for kt in range(KT):
                        tps = sm_psum.tile([128, 128], F16, name="smph")
                        T.transpose(tps[:], er[:, kt * 128:(kt + 1) * 128],
                                    identh[:])
                        V.tensor_copy(out=attT[kt][:, it * 128:
                                                   (it + 1) * 128],
                                      in_=tps[:])

                # ---- out[c,j] = sum_k pv[c,k] att[j,k] (fp16) ----
                for ct in range(CT):
                    ps = mm_psum.tile([128, W], F32, name="mm_ps")
                    for jc in range(2):
                        for kt in range(KT):
                            T.matmul(ps[:, jc * 512:(jc + 1) * 512],
                                     lhsT=pvT[kt][:, ct * 128:
                                                 (ct + 1) * 128],
                                     rhs=attT[kt][:, jc * 512:
                                                 (jc + 1) * 512],
                                     start=(kt == 0), stop=(kt == KT - 1))
                    t = pscr.tile([128, W], F32, name="psA")
                    S.activation(t[:], ps[:], AF.Identity)
                    nc.sync.dma_start(
                        out=dram["y"].ap()[s, ct * 128:(ct + 1) * 128, :],
                        in_=t[:])


def _build():
    nc = bass.Bass("TRN2", target_bir_lowering=False, debug=False,
                   num_devices=N_CORES)
    dram = {}
    for nm in ["x1", "x2"]:
        dram[nm] = nc.dram_tensor(nm, [SPC, C, W], F16,
                                  kind="ExternalInput")
    for nm in ["rw11T", "rw12T", "rw21T", "rw22T"]:
        dram[nm] = nc.dram_tensor(nm, [128, 6144], F16,
                                  kind="ExternalInput")
    for nm in ["qw1T", "qw2T", "kw1T", "kw2T", "vwT"]:
        dram[nm] = nc.dram_tensor(nm, [128, 2048], F16,
                                  kind="ExternalInput")
    for nm in ["qb", "kb", "vb", "rb1", "rb2"]:
        dram[nm] = nc.dram_tensor(nm, [C], F32, kind="ExternalInput")
    dram["y"] = nc.dram_tensor("y", [SPC, C, W], F32,
                               kind="ExternalOutput")
    for nm in ["xg_1", "xg_2", "g_1", "g_2"]:
        dram[nm] = nc.dram_tensor(nm, [SPC, C, W], F16)
    dram["cc_in"] = nc.dram_tensor("cc_in", [1, 4], F32)
    dram["cc_out"] = nc.dram_tensor("cc_out", [1, 4], F32,
                                    addr_space="Shared")

    with tile.TileContext(nc) as tc:
        with ExitStack() as ctx:
            _emit(nc, tc, ctx, dram)
    _split_multiwait(nc)
    return nc


_NC_CACHE = {}


def prepare_in_maps(inputs):
    x = np.ascontiguousarray(np.asarray(inputs["x"], dtype=np.float32))
    x1 = x.astype(np.float16)
    x2 = (x - x1.astype(np.float32)).astype(np.float16)

    def convT(w):
        # [co, ci, k] -> [ci_p, k, ci_t, co_t, co_l] flat [128, 6144]
        t = np.asarray(w, np.float32).transpose(1, 2, 0)
        t = t.reshape(CT, 128, 3, CT, 128).transpose(1, 2, 0, 3, 4)
        return np.ascontiguousarray(t.reshape(128, 6144))

    def oneT(w):
        # [co, ci, 1] -> [ci_p, ci_t, co] flat [128, 2048]
        t = np.asarray(w, np.float32)[:, :, 0].T
        t = t.reshape(CT, 128, C).transpose(1, 0, 2)
        return np.ascontiguousarray(t.reshape(128, CT * C))

    common = {}
    for nm, fT in (("rw1", convT), ("rw2", convT), ("qw", oneT),
                   ("kw", oneT)):
        wt = fT(inputs[nm])
        w1 = wt.astype(np.float16)
        common[f"{nm}1T"] = w1
        common[f"{nm}2T"] = (wt - w1.astype(np.float32)).astype(np.float16)
    common["vwT"] = oneT(inputs["vw"]).astype(np.float16)
    for nm in ["qb", "kb", "vb", "rb1", "rb2"]:
        common[nm] = np.ascontiguousarray(
            np.asarray(inputs[nm], dtype=np.float32))

    in_maps = []
    for core in range(N_CORES):
        m = dict(common)
        m["x1"] = np.ascontiguousarray(x1[core * SPC:(core + 1) * SPC])
        m["x2"] = np.ascontiguousarray(x2[core * SPC:(core + 1) * SPC])
        in_maps.append(m)
    return in_maps


def kernel(**inputs):
    if "nc" not in _NC_CACHE:
        _NC_CACHE["nc"] = _build()
    nc = _NC_CACHE["nc"]
    in_maps = prepare_in_maps(inputs)
    res = run_bass_kernel_spmd(nc, in_maps, core_ids=list(range(N_CORES)))
    y = np.concatenate([r["y"] for r in res.results], axis=0)
    return y


# revision 47
# speedup vs baseline: 1.0657x; 1.0657x over previous
"""Trainium2 Bass kernel for nn_CCAR_11579231830663 (dense_transformer).

Data-parallel over batch: 16 samples -> 8 NeuronCores x 2 samples. Only the
global z-score stats of x_g and g cross cores (AllReduce of 4 scalar sums).

Precision scheme: the softmax over energy is a near-argmax (energy scale ~1e4,
winner gaps down to ~0.1), so every matmul feeding the energy needs ~fp32
accuracy, but hardware fp32 matmul costs 4 cycles/row. Instead every operand
is kept as an fp16 limb pair: A1 = fp16(A), A2 = fp16(A - A1). fp16 carries
11 significant bits, so A@B ~= A1@B1 + A1@B2 + A2@B1 gives ~2^-21 relative
accuracy from three 1-cycle/row fp16 passes: 4x faster than fp32 mode with
negligible precision loss for this problem. The pv/attention output path
tolerates a single fp16 pass. Weights and x are pre-split and pre-transposed
on the host; x_g and g limb pairs spill to DRAM between the residual phase
and the attention phase.

Energy algebra (z-score folded out): E = a*pq^T (xc gc^T) pk with
a = 1/(s_xg*s_g) folded into the softmax exp. MT = (g xg^T) is computed on
UNCENTERED data; the centering flows into Mp = MT^T pk as rank-1 corrections
(means x row/col sums of pk) applied at PSUM eviction, so only the final
softmax waits on the AllReduce.
"""
import sys
sys.path.insert(0, '/opt/trn_rl_repo')

import numpy as np
from contextlib import ExitStack

import concourse.bass as bass
import concourse.tile as tile
from concourse import mybir
from concourse.masks import make_identity
from concourse.bass_utils import run_bass_kernel_spmd

F32 = mybir.dt.float32
F16 = mybir.dt.float16
AF = mybir.ActivationFunctionType
ALU = mybir.AluOpType
AX = mybir.AxisListType

N_CORES = 8
B, C, W = 16, 512, 1024
SPC = B // N_CORES      # samples per core
CT = C // 128           # channel tiles
KT = W // 128           # width 128-tiles
EPS = 1e-5
NTOT = float(B * C * W)

MAGIC = 12582912.0       # 1.5*2^23 fp32 round-to-nearest-int magic
TWOPI = float(2 * np.pi)
INV2PI = float(1.0 / (2 * np.pi))

# ---------------------------------------------------------------------------
# antenv.axon_hooks is missing in this container; run_bass_kernel_spmd
# imports it when tracing is requested. Provide a stub.
import types as _types

if 'antenv.axon_hooks' not in sys.modules:
    _m = _types.ModuleType('antenv.axon_hooks')
    _h = [None]
    _m.set_axon_ntff_profile_hook = lambda h: _h.__setitem__(0, h)
    _m.get_axon_ntff_profile_hook = lambda: _h[0]
    sys.modules['antenv.axon_hooks'] = _m
    try:
        import antenv as _antenv
        _antenv.axon_hooks = _m
    except ImportError:
        pass

# ---------------------------------------------------------------------------
# walrus workaround: limited sync waits per instruction; split excess waits
# onto same-engine NOPs.
_uid = [0]


def _split_multiwait(nc, limit=1):
    for f in nc.m.functions:
        for bb in f.blocks:
            insts = list(bb.instructions)
            out = []
            changed = False
            for inst in insts:
                si = inst.sync_info
                waits = list(si.on_wait) if si is not None and si.on_wait \
                    else []
                if len(waits) > limit:
                    changed = True
                    excess, keep = waits[:-limit], waits[-limit:]
                    si.on_wait = keep
                    inst.sync_info = si
                    for i in range(0, len(excess), limit):
                        chunk = excess[i:i + limit]
                        _uid[0] += 1
                        nop = mybir.InstNoOp(
                            name=f"I-waitsplit-{_uid[0]}", ins=[], outs=[])
                        nop.engine = inst.engine
                        nop.sync_info = mybir.SyncInfo(
                            on_wait=chunk, on_update=[])
                        out.append(nop)
                out.append(inst)
            if changed:
                bb.instructions = out


# ---------------------------------------------------------------------------
def _emit(nc, tc, ctx, dram):
    V = nc.vector
    S = nc.scalar
    T = nc.tensor

    singles = ctx.enter_context(tc.tile_pool(name="singles", bufs=1))
    spool = ctx.enter_context(tc.tile_pool(name="spool", bufs=1))
    nrm = ctx.enter_context(tc.tile_pool(name="nrm", bufs=2))
    mm_psum = ctx.enter_context(
        tc.tile_pool(name="mm_psum", bufs=3, space="PSUM"))
    sm_psum = ctx.enter_context(
        tc.tile_pool(name="sm_psum", bufs=2, space="PSUM"))

    ident = singles.tile([128, 128], F32, name="ident")
    make_identity(nc, ident[:])
    identh = singles.tile([128, 128], F16, name="identh")
    V.tensor_copy(out=identh[:], in_=ident[:])

    ones128 = singles.tile([128, 1], F32, name="ones128")
    V.memset(ones128[:], 1.0)
    onesh = singles.tile([128, 1], F16, name="onesh")
    V.tensor_copy(out=onesh[:], in_=ones128[:])
    onek1f = singles.tile([1, 128], F32, name="onek1f")
    V.memset(onek1f[:], 1.0)
    onek1 = singles.tile([1, 128], F16, name="onek1")
    V.tensor_copy(out=onek1[:], in_=onek1f[:])

    zcol2 = singles.tile([128, 2], F32, name="zcol2")
    V.memset(zcol2[:], 0.0)
    negmagic = singles.tile([128, 1], F32, name="negmagic")
    V.memset(negmagic[:], -MAGIC)
    inv2pic = singles.tile([128, 1], F32, name="inv2pic")
    V.memset(inv2pic[:], INV2PI)

    def load_bias_cols(name):
        t = singles.tile([128, CT], F32, name=f"{name}_cols")
        src = dram[name].ap().rearrange("(t p) -> p t", p=128)
        nc.sync.dma_start(out=t[:], in_=src)
        return t

    rb1b = load_bias_cols("rb1")
    rb2b = load_bias_cols("rb2")
    qbb = load_bias_cols("qb")
    kbb = load_bias_cols("kb")

    # stats columns: 0 sum_xg, 1 sumsq_xg, 2 sum_g, 3 sumsq_g
    stats_block = spool.tile([128, 4], F32, name="stats_block")
    V.memset(stats_block[:], 0.0)

    rsxg = [spool.tile([128, CT], F32, name=f"rsxg_{s}")
            for s in range(SPC)]
    rsg = [spool.tile([128, CT], F32, name=f"rsg_{s}")
           for s in range(SPC)]

    def rstd_from_var(varcol):
        veps = nrm.tile([128, 1], F32, name="veps")
        V.tensor_scalar_add(out=veps[:], in0=varcol, scalar1=EPS)
        s0 = nrm.tile([128, 1], F32, name="s0")
        S.activation(s0[:], veps[:], AF.Sqrt)
        y0 = nrm.tile([128, 1], F32, name="y0")
        V.reciprocal(out=y0[:], in_=s0[:])
        t1 = nrm.tile([128, 1], F32, name="nt1")
        V.tensor_tensor(out=t1[:], in0=y0[:], in1=y0[:], op=ALU.mult)
        V.tensor_tensor(out=t1[:], in0=t1[:], in1=veps[:], op=ALU.mult)
        V.tensor_scalar(out=t1[:], in0=t1[:], scalar1=-0.5, scalar2=1.5,
                        op0=ALU.mult, op1=ALU.add)
        y1 = nrm.tile([128, 1], F32, name="ny1")
        V.tensor_tensor(out=y1[:], in0=y0[:], in1=t1[:], op=ALU.mult)
        return y1

    # ======================= R phase: residual block =======================
    with ExitStack() as rctx:
        cw = rctx.enter_context(tc.tile_pool(name="cw", bufs=1))
        padp = rctx.enter_context(tc.tile_pool(name="padp", bufs=8))
        rscr = rctx.enter_context(tc.tile_pool(name="rscr", bufs=2))
        rhf = rctx.enter_context(tc.tile_pool(name="rhf", bufs=2))

        rw = {}
        for cn in ("rw1", "rw2"):
            for li in ("1", "2"):
                rw[cn + li] = cw.tile([128, 6144], F16, name=cn + li)
                nc.sync.dma_start(out=rw[cn + li][:],
                                  in_=dram[cn + li + "T"].ap())

        def wsl(wt, k, ci, co):
            base = ((k * CT + ci) * CT + co) * 128
            return wt[:, base:base + 128]

        def conv3(dst_cb, src1, src2, cn):
            """3-pass conv: w1*x1 + w1*x2 + w2*x1; dst_cb(co_t, ps)."""
            for co_t in range(CT):
                ps = mm_psum.tile([128, W], F32, name="mm_ps")
                for jc in range(2):
                    idx = 0
                    for wt, xt in ((rw[cn + "1"], src1),
                                   (rw[cn + "1"], src2),
                                   (rw[cn + "2"], src1)):
                        for k in range(3):
                            for ci_t in range(CT):
                                T.matmul(ps[:, jc * 512:(jc + 1) * 512],
                                         lhsT=wsl(wt, k, ci_t, co_t),
                                         rhs=xt[ci_t][:, jc * 512 + k + 1:
                                                      jc * 512 + k + 513],
                                         start=(idx == 0), stop=(idx == 35))
                                idx += 1
                dst_cb(co_t, ps)

        def inorm_sin(ps, bias_col, sin_dst, sin_accum=None):
            """sin_dst <- sin(instance_norm(ps + bias))."""
            t = rscr.tile([128, W], F32, name="rsA")
            S.activation(t[:], ps[:], AF.Identity, bias=bias_col)
            st = nrm.tile([128, 2, 6], F32, name="bn_st")
            V.bn_stats(st[:, 0, :], t[:, 0:512])
            V.bn_stats(st[:, 1, :], t[:, 512:1024])
            mv = nrm.tile([128, 2], F32, name="bn_mv")
            V.bn_aggr(mv[:], st[:])
            rstd = rstd_from_var(mv[:, 1:2])
            nmr = nrm.tile([128, 1], F32, name="nmr")
            V.tensor_tensor(out=nmr[:], in0=mv[:, 0:1], in1=rstd[:],
                            op=ALU.mult)
            V.tensor_scalar_mul(out=nmr[:], in0=nmr[:], scalar1=-1.0)
            w = rscr.tile([128, W], F32, name="rsB")
            S.activation(w[:], t[:], AF.Identity, bias=nmr[:],
                         scale=rstd[:])
            u = rscr.tile([128, W], F32, name="rsA")
            S.activation(u[:], w[:], AF.Identity, bias=negmagic[:],
                         scale=inv2pic[:])
            V.tensor_scalar_add(out=u[:], in0=u[:], scalar1=MAGIC)
            V.scalar_tensor_tensor(out=u[:], in0=u[:], scalar=-TWOPI,
                                   in1=w[:], op0=ALU.mult, op1=ALU.add)
            S.activation(sin_dst, u[:], AF.Sin, accum_out=sin_accum)

        for s in range(SPC):
            xp1 = [padp.tile([128, W + 4], F16, name="pad1")
                   for _ in range(CT)]
            xp2 = [padp.tile([128, W + 4], F16, name="pad2")
                   for _ in range(CT)]
            for c in range(CT):
                for t_ in (xp1[c], xp2[c]):
                    V.tensor_copy(out=t_[:, 0:2], in_=zcol2[:])
                    V.tensor_copy(out=t_[:, W + 2:W + 4], in_=zcol2[:])
                nc.sync.dma_start(
                    out=xp1[c][:, 2:W + 2],
                    in_=dram["x1"].ap()[s, c * 128:(c + 1) * 128, :])
                nc.sync.dma_start(
                    out=xp2[c][:, 2:W + 2],
                    in_=dram["x2"].ap()[s, c * 128:(c + 1) * 128, :])

            g11 = [padp.tile([128, W + 4], F16, name="pad1")
                   for _ in range(CT)]
            g12 = [padp.tile([128, W + 4], F16, name="pad2")
                   for _ in range(CT)]
            for c in range(CT):
                for t_ in (g11[c], g12[c]):
                    V.tensor_copy(out=t_[:, 0:2], in_=zcol2[:])
                    V.tensor_copy(out=t_[:, W + 2:W + 4], in_=zcol2[:])

            def c1_cb(co_t, ps):
                sf = rscr.tile([128, W], F32, name="rsB")
                inorm_sin(ps, rb1b[:, co_t:co_t + 1], sf[:])
                V.tensor_copy(out=g11[co_t][:, 2:W + 2], in_=sf[:])
                V.tensor_tensor(out=g12[co_t][:, 2:W + 2], in0=sf[:],
                                in1=g11[co_t][:, 2:W + 2],
                                op=ALU.subtract)

            conv3(c1_cb, xp1, xp2, "rw1")

            def c2_cb(co_t, ps):
                sl = slice(co_t * 128, (co_t + 1) * 128)
                sf = rscr.tile([128, W], F32, name="rsB")
                inorm_sin(ps, rb2b[:, co_t:co_t + 1], sf[:],
                          sin_accum=rsg[s][:, co_t:co_t + 1])
                # spill g limb pair to DRAM
                gp1 = rhf.tile([128, W], F16, name="gp1")
                V.tensor_copy(out=gp1[:], in_=sf[:])
                nc.sync.dma_start(out=dram["g_1"].ap()[s, sl, :],
                                  in_=gp1[:])
                gp2 = rhf.tile([128, W], F16, name="gp2")
                V.tensor_tensor(out=gp2[:], in0=sf[:], in1=gp1[:],
                                op=ALU.subtract)
                nc.sync.dma_start(out=dram["g_2"].ap()[s, sl, :],
                                  in_=gp2[:])
                gs2 = nrm.tile([128, 1], F32, name="gs2")
                sq = rscr.tile([128, W], F32, name="rsA")
                V.scalar_tensor_tensor(out=sq[:], in0=sf[:], scalar=0.0,
                                       in1=sf[:], op0=ALU.add,
                                       op1=ALU.mult, accum_out=gs2[:])
                # x_g = (x1 + x2) + g ; spill its limb pair
                xt = rscr.tile([128, W], F32, name="rsA")
                V.tensor_tensor(out=xt[:], in0=xp1[co_t][:, 2:W + 2],
                                in1=xp2[co_t][:, 2:W + 2], op=ALU.add)
                V.scalar_tensor_tensor(out=xt[:], in0=xt[:], scalar=0.0,
                                       in1=sf[:], op0=ALU.add, op1=ALU.add,
                                       accum_out=rsxg[s][:, co_t:co_t + 1])
                xs2 = nrm.tile([128, 1], F32, name="xs2")
                sq2 = rscr.tile([128, W], F32, name="rsB")
                V.scalar_tensor_tensor(out=sq2[:], in0=xt[:], scalar=0.0,
                                       in1=xt[:], op0=ALU.add,
                                       op1=ALU.mult, accum_out=xs2[:])
                xg1 = rhf.tile([128, W], F16, name="gp1")
                V.tensor_copy(out=xg1[:], in_=xt[:])
                nc.sync.dma_start(out=dram["xg_1"].ap()[s, sl, :],
                                  in_=xg1[:])
                xg2 = rhf.tile([128, W], F16, name="gp2")
                V.tensor_tensor(out=xg2[:], in0=xt[:], in1=xg1[:],
                                op=ALU.subtract)
                nc.sync.dma_start(out=dram["xg_2"].ap()[s, sl, :],
                                  in_=xg2[:])
                # global stats accumulation
                V.tensor_tensor(out=stats_block[:, 0:1],
                                in0=stats_block[:, 0:1],
                                in1=rsxg[s][:, co_t:co_t + 1], op=ALU.add)
                V.tensor_tensor(out=stats_block[:, 1:2],
                                in0=stats_block[:, 1:2], in1=xs2[:],
                                op=ALU.add)
                V.tensor_tensor(out=stats_block[:, 2:3],
                                in0=stats_block[:, 2:3],
                                in1=rsg[s][:, co_t:co_t + 1], op=ALU.add)
                V.tensor_tensor(out=stats_block[:, 3:4],
                                in0=stats_block[:, 3:4], in1=gs2[:],
                                op=ALU.add)

            conv3(c2_cb, g11, g12, "rw2")

    # ====================== AllReduce of the 4 sums ========================
    ps4 = mm_psum.tile([128, W], F32, name="mm_ps")
    T.matmul(ps4[:1, 0:4], lhsT=ones128[:], rhs=stats_block[:],
             start=True, stop=True)
    cc_sb = spool.tile([1, 4], F32, name="cc_sb")
    V.tensor_copy(out=cc_sb[:], in_=ps4[:1, 0:4])
    nc.sync.dma_start(out=dram["cc_in"].ap(), in_=cc_sb[:])
    nc.gpsimd.collective_compute(
        "AllReduce", ALU.add,
        replica_groups=[list(range(N_CORES))],
        ins=[dram["cc_in"].ap()],
        outs=[dram["cc_out"].ap()],
    )
    gstat = spool.tile([128, 4], F32, name="gstat")
    bcast = bass.AP(tensor=dram["cc_out"], offset=0, ap=[[0, 128], [1, 4]])
    nc.sync.dma_start(out=gstat[:], in_=bcast)

    def mean_rs(s1col, s2col, tag):
        m = spool.tile([128, 1], F32, name=f"m_{tag}")
        V.tensor_scalar_mul(out=m[:], in0=s1col, scalar1=1.0 / NTOT)
        t = spool.tile([128, 1], F32, name=f"v_{tag}")
        V.tensor_tensor(out=t[:], in0=s1col, in1=m[:], op=ALU.mult)
        V.tensor_scalar_mul(out=t[:], in0=t[:], scalar1=-1.0)
        V.tensor_tensor(out=t[:], in0=t[:], in1=s2col, op=ALU.add)
        V.tensor_scalar_mul(out=t[:], in0=t[:], scalar1=1.0 / (NTOT - 1.0))
        sq = spool.tile([128, 1], F32, name=f"sq_{tag}")
        S.activation(sq[:], t[:], AF.Sqrt)
        y0 = spool.tile([128, 1], F32, name=f"y0_{tag}")
        V.reciprocal(out=y0[:], in_=sq[:])
        t2 = spool.tile([128, 1], F32, name=f"t2_{tag}")
        V.tensor_tensor(out=t2[:], in0=y0[:], in1=y0[:], op=ALU.mult)
        V.tensor_tensor(out=t2[:], in0=t2[:], in1=t[:], op=ALU.mult)
        V.tensor_scalar(out=t2[:], in0=t2[:], scalar1=-0.5, scalar2=1.5,
                        op0=ALU.mult, op1=ALU.add)
        V.tensor_tensor(out=t2[:], in0=y0[:], in1=t2[:], op=ALU.mult)
        return m, t2

    m_xg, rs_xg = mean_rs(gstat[:, 0:1], gstat[:, 1:2], "xg")
    m_g, rs_g = mean_rs(gstat[:, 2:3], gstat[:, 3:4], "g")
    alpha = spool.tile([128, 1], F32, name="alpha")
    V.tensor_tensor(out=alpha[:], in0=rs_xg[:], in1=rs_g[:], op=ALU.mult)
    negalpha = spool.tile([128, 1], F32, name="negalpha")
    V.tensor_scalar_mul(out=negalpha[:], in0=alpha[:], scalar1=-1.0)
    negmg = spool.tile([128, 1], F32, name="negmg")
    V.tensor_scalar_mul(out=negmg[:], in0=m_g[:], scalar1=-1.0)
    negmx = spool.tile([128, 1], F32, name="negmx")
    V.tensor_scalar_mul(out=negmx[:], in0=m_xg[:], scalar1=-1.0)
    wmxmg = spool.tile([128, 1], F32, name="wmxmg")
    V.tensor_tensor(out=wmxmg[:], in0=m_xg[:], in1=m_g[:], op=ALU.mult)
    V.tensor_scalar_mul(out=wmxmg[:], in0=wmxmg[:], scalar1=float(W))

    # =================== P/M/E phases, one sample at a time ================
    with ExitStack() as ectx:
        pscr = ectx.enter_context(tc.tile_pool(name="pscr", bufs=2))
        psing = ectx.enter_context(tc.tile_pool(name="psing", bufs=1))
        vb_bc = psing.tile([128, C], F32, name="vb_bc")
        nc.sync.dma_start(out=vb_bc[:],
                          in_=bass.AP(tensor=dram["vb"], offset=0,
                                      ap=[[0, 128], [1, C]]))
        qk = psing.tile([128, 4 * 2048], F16, name="qkw")
        for i, nm in enumerate(["qw1T", "qw2T", "kw1T", "kw2T"]):
            nc.sync.dma_start(out=qk[:, i * 2048:(i + 1) * 2048],
                              in_=dram[nm].ap())
        vwt = psing.tile([128, 2048], F16, name="vwt")
        nc.sync.dma_start(out=vwt[:], in_=dram["vwT"].ap())

        def qsl(i, ci, co):
            base = i * 2048 + ci * 512 + co * 128
            return qk[:, base:base + 128]

        def split_pair(src_ap, l1, l2):
            """l1 <- fp16(src); l2 <- fp16(src - l1)."""
            V.tensor_copy(out=l1, in_=src_ap)
            V.tensor_tensor(out=l2, in0=src_ap, in1=l1, op=ALU.subtract)

        for s in range(SPC):
            with ExitStack() as sctx:
                # right-side stack: pools dying mid-sample
                hctx = sctx.enter_context(ExitStack())
                mtp = hctx.enter_context(
                    tc.tile_pool(name="mtp", bufs=1, side="right"))
                pkp = hctx.enter_context(
                    tc.tile_pool(name="pkp", bufs=1, side="right"))

                MT_1, MT_2 = [], []
                pq_1, pq_2 = [], []
                pk_1, pk_2 = [], []
                pvT = []
                pqp = sctx.enter_context(tc.tile_pool(name="pqp", bufs=1))
                pvp = sctx.enter_context(tc.tile_pool(name="pvp", bufs=1))
                with ExitStack() as tctx:
                    trio = tctx.enter_context(
                        tc.tile_pool(name="trio", bufs=1, side="right"))
                    xg_1, xg_2, g_1, g_2 = [], [], [], []
                    for dname, l1s, l2s, tag in (
                            ("xg", xg_1, xg_2, "x"), ("g", g_1, g_2, "g")):
                        for c in range(CT):
                            sl = slice(c * 128, (c + 1) * 128)
                            for li, ls in ((1, l1s), (2, l2s)):
                                t = trio.tile([128, W], F16,
                                              name=f"{tag}{li}_{c}")
                                nc.sync.dma_start(
                                    out=t[:],
                                    in_=dram[f"{dname}_{li}"]
                                    .ap()[s, sl, :])
                                ls.append(t)

                    # ---- transposed limbs, then MT (uncentered) ----
                    t2ctx = tctx.enter_context(ExitStack())
                    tp = t2ctx.enter_context(tc.tile_pool(name="tp",
                                                          bufs=1))

                    def transposed(src1, src2, tag):
                        t1s, t2s = [], []
                        for kt in range(KT):
                            ksl = slice(kt * 128, (kt + 1) * 128)
                            t1 = tp.tile([128, C], F16, name=f"{tag}1{kt}")
                            t2 = tp.tile([128, C], F16, name=f"{tag}2{kt}")
                            for ci in range(CT):
                                csl = slice(ci * 128, (ci + 1) * 128)
                                ps = sm_psum.tile([128, 128], F16,
                                                  name="smph")
                                T.transpose(ps[:], src1[ci][:, ksl],
                                            identh[:])
                                V.tensor_copy(out=t1[:, csl], in_=ps[:])
                                ps2 = sm_psum.tile([128, 128], F16,
                                                   name="smph")
                                T.transpose(ps2[:], src2[ci][:, ksl],
                                            identh[:])
                                V.tensor_copy(out=t2[:, csl], in_=ps2[:])
                            t1s.append(t1)
                            t2s.append(t2)
                        return t1s, t2s

                    xgT_1, xgT_2 = transposed(xg_1, xg_2, "xT")
                    ggT_1, ggT_2 = transposed(g_1, g_2, "gT")

                    for cpt in range(CT):
                        ps = mm_psum.tile([128, W], F32, name="mm_ps")
                        idx = 0
                        for (gt, xt) in ((ggT_1, xgT_1), (ggT_1, xgT_2),
                                         (ggT_2, xgT_1)):
                            for kt in range(KT):
                                T.matmul(
                                    ps[:, 0:C],
                                    lhsT=gt[kt][:, cpt * 128:
                                                (cpt + 1) * 128],
                                    rhs=xt[kt][:],
                                    start=(idx == 0), stop=(idx == 23))
                                idx += 1
                        l1 = mtp.tile([128, C], F16, name=f"MT1{cpt}")
                        l2 = mtp.tile([128, C], F16, name=f"MT2{cpt}")
                        split_pair(ps[:, 0:C], l1[:], l2[:])
                        MT_1.append(l1)
                        MT_2.append(l2)
                    t2ctx.close()  # free transposed limbs; keep naturals

                    # ---- pq / pk (3-pass projections), pvT (fp16) ----
                    def proj3(iw, src1, src2, bias_cols, prefix, pool,
                              o1, o2):
                        for co_t in range(CT):
                            ps = mm_psum.tile([128, W], F32, name="mm_ps")
                            for jc in range(2):
                                idx = 0
                                for wi, xt in ((iw, src1), (iw, src2),
                                               (iw + 1, src1)):
                                    for ci_t in range(CT):
                                        T.matmul(
                                            ps[:, jc * 512:(jc + 1) * 512],
                                            lhsT=qsl(wi, ci_t, co_t),
                                            rhs=xt[ci_t][:, jc * 512:
                                                         (jc + 1) * 512],
                                            start=(idx == 0),
                                            stop=(idx == 11))
                                        idx += 1
                            t = pscr.tile([128, W], F32, name="psA")
                            S.activation(t[:], ps[:], AF.Identity,
                                         bias=bias_cols[:, co_t:co_t + 1])
                            l1 = pool.tile([128, W], F16,
                                           name=f"{prefix}1{co_t}")
                            l2 = pool.tile([128, W], F16,
                                           name=f"{prefix}2{co_t}")
                            split_pair(t[:], l1[:], l2[:])
                            o1.append(l1)
                            o2.append(l2)

                    proj3(0, xg_1, xg_2, qbb, "pq", pqp, pq_1, pq_2)
                    proj3(2, g_1, g_2, kbb, "pk", pkp, pk_1, pk_2)
                    for kt in range(KT):
                        ps = mm_psum.tile([128, W], F32, name="mm_ps")
                        for ci_t in range(CT):
                            T.matmul(
                                ps[:, 0:C],
                                lhsT=g_1[ci_t][:, kt * 128:(kt + 1) * 128],
                                rhs=vwt[:, ci_t * 512:(ci_t + 1) * 512],
                                start=(ci_t == 0), stop=(ci_t == CT - 1))
                        t = pvp.tile([128, C], F16, name=f"pvT{kt}")
                        V.scalar_tensor_tensor(
                            out=t[:], in0=ps[:, 0:C], scalar=0.0,
                            in1=vb_bc[:], op0=ALU.add, op1=ALU.add)
                        pvT.append(t)
                # natural limb pairs freed here

                # ---- Mp = MT^T pk + rank-1 centering corrections ----
                # cs[j] = colsum pk ; u[j] = sum_c' rsg[c'] pk[c',j]
                Mp_1, Mp_2 = [], []
                mpp = sctx.enter_context(tc.tile_pool(name="mpp", bufs=1))
                with ExitStack() as mctx:
                    mrow = mctx.enter_context(
                        tc.tile_pool(name="mrow", bufs=1, side="right"))
                    rsch = []
                    for cpt in range(CT):
                        rc = mrow.tile([128, 1], F16, name=f"rsch{cpt}")
                        V.tensor_copy(out=rc[:],
                                      in_=rsg[s][:, cpt:cpt + 1])
                        rsch.append(rc)
                    psr = mm_psum.tile([128, W], F32, name="mm_ps")
                    psu = mm_psum.tile([128, W], F32, name="mm_ps")
                    for jc in range(2):
                        sl = slice(jc * 512, (jc + 1) * 512)
                        idx = 0
                        for pkt in (pk_1, pk_2):
                            for cpt in range(CT):
                                T.matmul(psr[:1, sl], lhsT=onesh[:],
                                         rhs=pkt[cpt][:, sl],
                                         start=(idx == 0), stop=(idx == 7))
                                T.matmul(psu[:1, sl],
                                         lhsT=rsch[cpt][:],
                                         rhs=pkt[cpt][:, sl],
                                         start=(idx == 0), stop=(idx == 7))
                                idx += 1
                    csrow = mrow.tile([1, W], F16, name="csrow")
                    V.tensor_copy(out=csrow[:], in_=psr[:1, :])
                    urow = mrow.tile([1, W], F16, name="urow")
                    V.tensor_copy(out=urow[:], in_=psu[:1, :])
                    # broadcast both rows to [128, W]
                    psb = mm_psum.tile([128, W], F32, name="mm_ps")
                    psb2 = mm_psum.tile([128, W], F32, name="mm_ps")
                    for jc in range(2):
                        sl = slice(jc * 512, (jc + 1) * 512)
                        T.matmul(psb[:, sl], lhsT=onek1[:],
                                 rhs=csrow[:, sl], start=True, stop=True)
                        T.matmul(psb2[:, sl], lhsT=onek1[:],
                                 rhs=urow[:, sl], start=True, stop=True)
                    cs_bc = mrow.tile([128, W], F32, name="cs_bc")
                    V.tensor_copy(out=cs_bc[:], in_=psb[:, :])
                    # vcomb = -m_x*u + W*m_x*m_g*cs (same for all rows)
                    vcomb = mrow.tile([128, W], F32, name="vcomb")
                    V.tensor_scalar(out=vcomb[:], in0=cs_bc[:],
                                    scalar1=wmxmg[:], scalar2=None,
                                    op0=ALU.mult, op1=ALU.bypass)
                    V.scalar_tensor_tensor(out=vcomb[:], in0=psb2[:, :],
                                           scalar=negmx[:], in1=vcomb[:],
                                           op0=ALU.mult, op1=ALU.add)

                    for ct in range(CT):
                        ps = mm_psum.tile([128, W], F32, name="mm_ps")
                        for jc in range(2):
                            idx = 0
                            for (mt, pkt) in ((MT_1, pk_1), (MT_1, pk_2),
                                              (MT_2, pk_1)):
                                for cpt in range(CT):
                                    T.matmul(
                                        ps[:, jc * 512:(jc + 1) * 512],
                                        lhsT=mt[cpt][:, ct * 128:
                                                    (ct + 1) * 128],
                                        rhs=pkt[cpt][:, jc * 512:
                                                    (jc + 1) * 512],
                                        start=(idx == 0), stop=(idx == 11))
                                    idx += 1
                        # Mp_c = ps - m_g*rsx[c]*cs + vcomb
                        ngr = nrm.tile([128, 1], F32, name="ngr")
                        V.tensor_tensor(out=ngr[:], in0=negmg[:],
                                        in1=rsxg[s][:, ct:ct + 1],
                                        op=ALU.mult)
                        t1 = pscr.tile([128, W], F32, name="psA")
                        V.tensor_scalar(out=t1[:], in0=cs_bc[:],
                                        scalar1=ngr[:], scalar2=None,
                                        op0=ALU.mult, op1=ALU.bypass)
                        V.tensor_tensor(out=t1[:], in0=t1[:],
                                        in1=vcomb[:], op=ALU.add)
                        V.tensor_tensor(out=t1[:], in0=t1[:],
                                        in1=ps[:], op=ALU.add)
                        l1 = mpp.tile([128, W], F16, name=f"Mp1{ct}")
                        l2 = mpp.tile([128, W], F16, name=f"Mp2{ct}")
                        split_pair(t1[:], l1[:], l2[:])
                        Mp_1.append(l1)
                        Mp_2.append(l2)

                # ---- energy -> softmax -> att^T (3-pass E) ----
                hctx.close()  # free MT + pk before E allocates attT
                attp = sctx.enter_context(tc.tile_pool(name="attp",
                                                       bufs=1))
                escr = sctx.enter_context(tc.tile_pool(name="escr",
                                                       bufs=2))
                attT = [attp.tile([128, W], F16, name=f"attT_{kt}")
                        for kt in range(KT)]
                for it in range(KT):
                    ps = mm_psum.tile([128, W], F32, name="mm_ps")
                    for jc in range(2):
                        idx = 0
                        for (pqt, mpt) in ((pq_1, Mp_1), (pq_1, Mp_2),
                                           (pq_2, Mp_1)):
                            for ct in range(CT):
                                T.matmul(
                                    ps[:, jc * 512:(jc + 1) * 512],
                                    lhsT=pqt[ct][:, it * 128:
                                                (it + 1) * 128],
                                    rhs=mpt[ct][:, jc * 512:
                                               (jc + 1) * 512],
                                    start=(idx == 0), stop=(idx == 11))
                                idx += 1
                    rowmax = nrm.tile([128, 1], F32, name="rowmax")
                    V.tensor_reduce(out=rowmax[:], in_=ps[:], axis=AX.X,
                                    op=ALU.max)
                    nb = nrm.tile([128, 1], F32, name="negb")
                    V.tensor_tensor(out=nb[:], in0=rowmax[:],
                                    in1=negalpha[:], op=ALU.mult)
                    e = pscr.tile([128, W], F32, name="psA")
                    rowsum = nrm.tile([128, 1], F32, name="rowsum")
                    S.activation(e[:], ps[:], AF.Exp, bias=nb[:],
                                 scale=alpha[:], accum_out=rowsum[:])
                    rs = nrm.tile([128, 1], F32, name="rs")
                    V.reciprocal(out=rs[:], in_=rowsum[:])
                    er = escr.tile([128, W], F16, name="psR")
                    V.tensor_scalar_mul(out=er[:], in0=e[:], scalar1=rs[:])
                    for kt in range(KT):
                        tps = sm_psum.tile([128, 128], F16, name="smph")
                        T.transpose(tps[:], er[:, kt * 128:(kt + 1) * 128],
                                    identh[:])
                        V.tensor_copy(out=attT[kt][:, it * 128:
                                                   (it + 1) * 128],
                                      in_=tps[:])

                # ---- out[c,j] = sum_k pv[c,k] att[j,k] (fp16) ----
                for ct in range(CT):
                    ps = mm_psum.tile([128, W], F32, name="mm_ps")
                    for jc in range(2):
                        for kt in range(KT):
                            T.matmul(ps[:, jc * 512:(jc + 1) * 512],
                                     lhsT=pvT[kt][:, ct * 128:
                                                 (ct + 1) * 128],
                                     rhs=attT[kt][:, jc * 512:
                                                 (jc + 1) * 512],
                                     start=(kt == 0), stop=(kt == KT - 1))
                    t = pscr.tile([128, W], F32, name="psA")
                    S.activation(t[:], ps[:], AF.Identity)
                    nc.sync.dma_start(
                        out=dram["y"].ap()[s, ct * 128:(ct + 1) * 128, :],
                        in_=t[:])


def _build():
    nc = bass.Bass("TRN2", target_bir_lowering=False, debug=False,
                   num_devices=N_CORES)
    dram = {}
    for nm in ["x1", "x2"]:
        dram[nm] = nc.dram_tensor(nm, [SPC, C, W], F16,
                                  kind="ExternalInput")
    for nm in ["rw11T", "rw12T", "rw21T", "rw22T"]:
        dram[nm] = nc.dram_tensor(nm, [128, 6144], F16,
                                  kind="ExternalInput")
    for nm in ["qw1T", "qw2T", "kw1T", "kw2T", "vwT"]:
        dram[nm] = nc.dram_tensor(nm, [128, 2048], F16,
                                  kind="ExternalInput")
    for nm in ["qb", "kb", "vb", "rb1", "rb2"]:
        dram[nm] = nc.dram_tensor(nm, [C], F32, kind="ExternalInput")
    dram["y"] = nc.dram_tensor("y", [SPC, C, W], F32,
                               kind="ExternalOutput")
    for nm in ["xg_1", "xg_2", "g_1", "g_2"]:
        dram[nm] = nc.dram_tensor(nm, [SPC, C, W], F16)
    dram["cc_in"] = nc.dram_tensor("cc_in", [1, 4], F32)
    dram["cc_out"] = nc.dram_tensor("cc_out", [1, 4], F32,
                                    addr_space="Shared")

    with tile.TileContext(nc) as tc:
        with ExitStack() as ctx:
            _emit(nc, tc, ctx, dram)
    _split_multiwait(nc)
    return nc


_NC_CACHE = {}


def prepare_in_maps(inputs):
    x = np.ascontiguousarray(np.asarray(inputs["x"], dtype=np.float32))
    x1 = x.astype(np.float16)
    x2 = (x - x1.astype(np.float32)).astype(np.float16)

    def convT(w):
        # [co, ci, k] -> [ci_p, k, ci_t, co_t, co_l] flat [128, 6144]
        t = np.asarray(w, np.float32).transpose(1, 2, 0)
        t = t.reshape(CT, 128, 3, CT, 128).transpose(1, 2, 0, 3, 4)
        return np.ascontiguousarray(t.reshape(128, 6144))

    def oneT(w):
        # [co, ci, 1] -> [ci_p, ci_t, co] flat [128, 2048]
        t = np.asarray(w, np.float32)[:, :, 0].T
        t = t.reshape(CT, 128, C).transpose(1, 0, 2)
        return np.ascontiguousarray(t.reshape(128, CT * C))

    common = {}
    for nm, fT in (("rw1", convT), ("rw2", convT), ("qw", oneT),
                   ("kw", oneT)):
        wt = fT(inputs[nm])
        w1 = wt.astype(np.float16)
        common[f"{nm}1T"] = w1
        common[f"{nm}2T"] = (wt - w1.astype(np.float32)).astype(np.float16)
    common["vwT"] = oneT(inputs["vw"]).astype(np.float16)
    for nm in ["qb", "kb", "vb", "rb1", "rb2"]:
        common[nm] = np.ascontiguousarray(
            np.asarray(inputs[nm], dtype=np.float32))

    in_maps = []
    for core in range(N_CORES):
        m = dict(common)
        m["x1"] = np.ascontiguousarray(x1[core * SPC:(core + 1) * SPC])
        m["x2"] = np.ascontiguousarray(x2[core * SPC:(core + 1) * SPC])
        in_maps.append(m)
    return in_maps


def kernel(**inputs):
    if "nc" not in _NC_CACHE:
        _NC_CACHE["nc"] = _build()
    nc = _NC_CACHE["nc"]
    in_maps = prepare_in_maps(inputs)
    res = run_bass_kernel_spmd(nc, in_maps, core_ids=list(range(N_CORES)))
    y = np.concatenate([r["y"] for r in res.results], axis=0)
    return y
